# revision 3
# baseline (speedup 1.0000x reference)
"""Trainium2 Bass kernel: K-step Euler rollout of a kinematic bicycle model.

Full inputs:
  initial_state [131072, 4] f32, controls [131072, 64, 2] f32,
  timestep scalar f32, agents_pars [131072, 2] f32
Output: [131072, 64, 4] f32 (state after each of the 64 steps).

Strategy: pure data parallel over 8 NeuronCores (16384 agents each).
Per core the rollout is decomposed into 4 segmented prefix scans on DVE
(tensor_tensor_scan, 0/1 bf16 mask resets state at agent boundaries):
    V: dt*vel scan (65-slot, bf16, slot0 = dt*vel0)
    W: yaw scan (65-slot, bf16 inputs, fp32 scan state)
    X/Y: position scans writing fp32 out lanes directly.
tan(steer) ~= steer*(steer^2+3)/3 (|steer|<0.3), the /3 folded into
1/(3L).  cos(yaw) = sin(pi/2-|yaw|) (|yaw|<pi).  Engine balance:
DVE scans + cheap bf16 TS/TT, Scalar activations + strided out-lane
copies, GpSimd the big elementwise muls + tiny seed ops.  Stages are
software-pipelined over 8 groups so each engine queue only contains
ops whose producers ran in earlier iterations.
"""
import os
import sys

for _p in ("/opt/trn_rl_repo", "/root/.axon_site/_ro/trn_rl_repo"):
    if os.path.isdir(_p) and _p not in sys.path:
        sys.path.insert(0, _p)

import numpy as np
import concourse.bass as bass
import concourse.bacc as bacc
import concourse.tile as tile
from concourse import mybir

F32 = mybir.dt.float32
BF16 = mybir.dt.bfloat16
AF = mybir.ActivationFunctionType
ALU = mybir.AluOpType

B = 131072
K = 64
NCORES = 8
BC = B // NCORES          # 16384 agents per core
P = 128                   # partitions
AG = 16                   # agents per partition per group
GRP = BC // (P * AG)      # 8 groups per core
PI = float(np.pi)

_cache = {}


def _build(dt: float):
    """Build the per-core SPMD program (identical on all 8 cores)."""
    nc = bacc.Bacc("TRN2", debug=False)

    d_aux = nc.dram_tensor("aux", [BC, 6], F32, kind="ExternalInput").ap()
    d_ctrl = nc.dram_tensor("controls", [BC, K, 2], F32, kind="ExternalInput").ap()
    d_out = nc.dram_tensor("out", [BC, K, 4], F32, kind="ExternalOutput").ap()

    r_aux = d_aux.rearrange("(g p a) c -> g p (a c)", g=GRP, p=P, a=AG)
    r_ctrl = d_ctrl.rearrange("(g p a) k c -> g p (a k c)", g=GRP, p=P, a=AG)
    r_out = d_out.rearrange("(g p a) k c -> g p (a k c)", g=GRP, p=P, a=AG)

    flat = lambda t: t.rearrange("p a k -> p (a k)")

    with tile.TileContext(nc) as tc:
        with (
            tc.tile_pool(name="consts", bufs=1) as consts,
            tc.tile_pool(name="io", bufs=2) as io,
            tc.tile_pool(name="mid", bufs=1) as mid,
        ):
            # constants
            mask65 = consts.tile([P, AG, 65], BF16)
            nc.vector.memset(mask65, 1.0)
            nc.vector.memset(mask65[:, :, 0], 0.0)
            mask64 = consts.tile([P, AG, 64], BF16)
            nc.vector.memset(mask64, 1.0)
            nc.vector.memset(mask64[:, :, 0], 0.0)
            c_dt2 = consts.tile([P, 1], F32)
            nc.vector.memset(c_dt2, dt * dt)
            c_pi2 = consts.tile([P, 1], F32)
            nc.vector.memset(c_pi2, PI / 2)
            c_m1 = consts.tile([P, 1], F32)
            nc.vector.memset(c_m1, -1.0)
            c_invdt = consts.tile([P, 1], F32)
            nc.vector.memset(c_invdt, 1.0 / dt)
            c_dtb = consts.tile([P, 1], F32)
            nc.vector.memset(c_dtb, dt)

            st = {}

            # ---- stage functions (g = group index) ----
            def s_load(g):
                ctrl_t = io.tile([P, AG, K, 2], F32, tag="ctrl", bufs=3, name=f"ctrl{g}")
                aux_t = io.tile([P, AG, 6], F32, tag="aux", bufs=8, name=f"aux{g}")
                nc.sync.dma_start(ctrl_t, r_ctrl[g])
                nc.sync.dma_start(aux_t, r_aux[g])
                st[g] = dict(ctrl=ctrl_t, aux=aux_t)

            def s_prep(g):
                d = st[g]
                ctrl_t, aux_t = d["ctrl"], d["aux"]
                steer = ctrl_t[:, :, :, 1]
                # Scalar: t165[1:65] = dt^2*accel ; q3 = steer^2
                t165 = mid.tile([P, AG, 65], BF16, tag="t165", bufs=7, name=f"t165_{g}")
                nc.scalar.activation(t165[:, :, 1:65], ctrl_t[:, :, :, 0], AF.Copy, scale=c_dt2)
                q3 = mid.tile([P, AG, K], BF16, tag="q3", bufs=3, name=f"q3_{g}")
                nc.scalar.activation(q3, steer, AF.Square)
                # DVE: invL3 = 1/(3L)
                t3L = mid.tile([P, AG], F32, tag="t3L", bufs=3, name=f"t3L{g}")
                nc.vector.tensor_scalar_mul(t3L, aux_t[:, :, 4], 3.0)
                invL3 = mid.tile([P, AG], F32, tag="invL3", bufs=4, name=f"invL3_{g}")
                nc.vector.reciprocal(invL3, t3L)
                # GpSimd: t165 slot0 = dt*vel0
                nc.gpsimd.tensor_tensor(
                    t165[:, :, 0], aux_t[:, :, 3], c_dtb.broadcast_to([P, AG]), ALU.mult
                )
                d.update(t165=t165, q3=q3, invL3=invL3)

            def s_scanv(g):
                d = st[g]
                nc.vector.tensor_tensor_scan(
                    flat(d["t165"]), flat(mask65), flat(d["t165"]), 0.0, ALU.mult, ALU.add
                )
                q3p3 = mid.tile([P, AG, K], BF16, tag="q3p3", bufs=3, name=f"q3p3_{g}")
                nc.vector.tensor_scalar_add(q3p3, d["q3"], 3.0)
                # GpSimd: sL = steer/(3L); last ctrl read, so ctrl needs 3 bufs
                sL = mid.tile([P, AG, K], BF16, tag="sL", bufs=3, name=f"sL{g}")
                nc.gpsimd.tensor_tensor(
                    sL, d["ctrl"][:, :, :, 1],
                    d["invL3"].unsqueeze(2).broadcast_to([P, AG, K]), ALU.mult
                )
                d.update(q3p3=q3p3, sL=sL)

            def s_mulw(g):
                d = st[g]
                m1 = mid.tile([P, AG, K], BF16, tag="m1", bufs=2, name=f"m1_{g}")
                nc.gpsimd.tensor_tensor(m1, d["t165"][:, :, 0:64], d["sL"], ALU.mult)
                d.update(m1=m1)

            def s_winw(g):
                d = st[g]
                w265 = mid.tile([P, AG, 65], BF16, tag="w265", bufs=6, name=f"w265_{g}")
                nc.vector.tensor_tensor(w265[:, :, 1:65], d["m1"], d["q3p3"], ALU.mult)
                nc.gpsimd.tensor_copy(w265[:, :, 0], d["aux"][:, :, 2])
                d.update(w265=w265)

            def s_scanw(g):
                d = st[g]
                nc.vector.tensor_tensor_scan(
                    flat(d["w265"]), flat(mask65), flat(d["w265"]), 0.0, ALU.mult, ALU.add
                )

            def s_trig(g):
                d = st[g]
                Yex = d["w265"][:, :, 0:64]
                sinY = mid.tile([P, AG, K], BF16, tag="sinY", bufs=3, name=f"sinY{g}")
                nc.scalar.activation(sinY, Yex, AF.Sin)
                absY = mid.tile([P, AG, K], BF16, tag="absY", bufs=2, name=f"absY{g}")
                nc.scalar.activation(absY, Yex, AF.Abs)
                cosY = mid.tile([P, AG, K], BF16, tag="cosY", bufs=3, name=f"cosY{g}")
                nc.scalar.activation(cosY, absY, AF.Sin, scale=c_m1, bias=c_pi2)
                d.update(sinY=sinY, cosY=cosY)

            def s_mulxy(g):
                d = st[g]
                Vex = d["t165"][:, :, 0:64]
                xin = mid.tile([P, AG, K], F32, tag="xin", bufs=3, name=f"xin{g}")
                nc.gpsimd.tensor_tensor(xin, Vex, d["cosY"], ALU.mult)
                nc.gpsimd.tensor_tensor(xin[:, :, 0], xin[:, :, 0], d["aux"][:, :, 0], ALU.add)
                yin = mid.tile([P, AG, K], F32, tag="yin", bufs=3, name=f"yin{g}")
                nc.gpsimd.tensor_tensor(yin, Vex, d["sinY"], ALU.mult)
                nc.gpsimd.tensor_tensor(yin[:, :, 0], yin[:, :, 0], d["aux"][:, :, 1], ALU.add)
                d.update(xin=xin, yin=yin)

            def s_scanxy(g):
                d = st.pop(g)
                out_t = io.tile([P, AG, K, 4], F32, tag="out", bufs=3, name=f"out{g}")
                xlane = out_t[:, :, :, 0].rearrange("p a k -> p (a k)")
                nc.vector.tensor_tensor_scan(
                    xlane, flat(mask64), flat(d["xin"]), 0.0, ALU.mult, ALU.add
                )
                ylane = out_t[:, :, :, 1].rearrange("p a k -> p (a k)")
                nc.vector.tensor_tensor_scan(
                    ylane, flat(mask64), flat(d["yin"]), 0.0, ALU.mult, ALU.add
                )
                nc.scalar.activation(out_t[:, :, :, 2], d["w265"][:, :, 1:65], AF.Copy)
                nc.scalar.activation(out_t[:, :, :, 3], d["t165"][:, :, 1:65], AF.Copy, scale=c_invdt)
                nc.sync.dma_start(r_out[g], out_t.rearrange("p a k c -> p (a k c)"))

            # deepest-first emission; group for stage with offset o at iter i is i-o
            stages = [
                (8, s_scanxy),
                (5, s_scanw),
                (6, s_trig),
                (7, s_mulxy),
                (4, s_winw),
                (3, s_mulw),
                (2, s_scanv),
                (1, s_prep),
                (0, s_load),
            ]
            for it in range(GRP + 8):
                for off, fn in stages:
                    g = it - off
                    if 0 <= g < GRP:
                        fn(g)

    nc.compile()
    return nc


def _get(dt: float):
    key = round(float(dt), 12)
    if key not in _cache:
        _cache[key] = _build(float(dt))
    return _cache[key]


def kernel(initial_state, controls, timestep, agents_pars, _trace=False):
    initial_state = np.ascontiguousarray(np.asarray(initial_state, dtype=np.float32))
    controls = np.ascontiguousarray(np.asarray(controls, dtype=np.float32))
    agents_pars = np.ascontiguousarray(np.asarray(agents_pars, dtype=np.float32))
    dt = float(np.asarray(timestep, dtype=np.float32))

    nc = _get(dt)
    aux = np.concatenate([initial_state, agents_pars], axis=1)
    in_maps = []
    for c in range(NCORES):
        s = slice(c * BC, (c + 1) * BC)
        in_maps.append({"aux": aux[s], "controls": controls[s]})
    from concourse import bass_utils

    r = bass_utils.run_bass_kernel_spmd(
        nc, in_maps, core_ids=list(range(NCORES)), trace=_trace
    )
    out = np.concatenate([r.results[c]["out"] for c in range(NCORES)], axis=0)
    if _trace:
        kernel.last_result = r
    return out


if __name__ == "__main__":
    # quick CoreSim check on one core's shard
    from concourse.bass_interp import CoreSim

    rng = np.random.default_rng(0)
    init = np.stack(
        [
            rng.normal(0, 10, BC),
            rng.normal(0, 10, BC),
            rng.normal(0, 0.5, BC),
            rng.normal(5, 2, BC),
        ],
        axis=-1,
    ).astype(np.float32)
    ctrl = (rng.standard_normal((BC, K, 2)) * np.array([1.0, 0.05])).astype(np.float32)
    pars = np.stack(
        [3 + 3 * rng.random(BC), 1.5 + rng.random(BC)], axis=-1
    ).astype(np.float32)
    dt = np.float32(0.1)

    nc = _get(float(dt))
    sim = CoreSim(nc, trace=False)
    sim.tensor("aux")[:] = np.concatenate([init, pars], axis=1)
    sim.tensor("controls")[:] = ctrl
    sim.simulate(check_with_hw=False)
    got = np.array(sim.tensor("out"))

    # numpy reference
    x, y, yaw, vel = (init[:, i].astype(np.float64) for i in range(4))
    L = pars[:, 0].astype(np.float64)
    exp = np.zeros((BC, K, 4))
    dtf = float(dt)
    for k in range(K):
        a = ctrl[:, k, 0].astype(np.float64)
        s = ctrl[:, k, 1].astype(np.float64)
        x = x + dtf * vel * np.cos(yaw)
        y = y + dtf * vel * np.sin(yaw)
        yaw = yaw + dtf * vel * np.tan(s) / L
        vel = vel + dtf * a
        exp[:, k] = np.stack([x, y, yaw, vel], axis=-1)
    err = np.linalg.norm(got - exp) / np.linalg.norm(exp)
    print("CoreSim relnorm vs numpy ref:", err)
    for c in range(4):
        e = np.abs(got[:, :, c] - exp[:, :, c]).max()
        print(f"  lane {c}: absmax {e:.3e}")
